# revision 36
# baseline (speedup 1.0000x reference)
"""Guide-token attention kernel for Trainium2 (8 NeuronCores).

Module: y[b] = softmax(((Q+tQ) @ (K+tK)^T)/sqrt(hd)) @ V  per head, where
  Q = x @ Wq^T + bq, K = x @ Wk^T + bk, V = x @ Wv^T + bv,
  tQ/tK are projections of a per-batch guide token (broadcast over seq).

Shapes: x [4, 1024, 1024], tokens [4, 1, 1024], W* [1024, 1024], b* [1024].
H=16 heads, hd=64.

Sharding: 8 cores = 4 batches x 2 head-groups (8 heads each); weights
column-sharded per head group; each core sees one batch -> no cross-core
communication.

Layout (PE contracts over the partition axis; no on-chip transposes):
  - host pre-transposes x[b] -> xT [D, S] and W slices (bf16), and
    precomputes the tiny guide-token adds (tq + 2*bq etc.).
  - QT/KT computed transposed [feat, S]; V computed natural [S, feat].
  - scores computed directly transposed per head: sT[k, q] = cK @ cQ^T
    (lhsT = cKT slice, rhs = cQT slice, contraction = hd = 64); the two
    heads of a pair live on PE row halves -> concurrent streams.
  - exp on ScalarE over two-bank PSUM tiles [128, 2, 512] -> bf16 probs.
    Softmax max-subtraction skipped: |scores| <= ~15, safe in fp32/bf16.
  - AV: lhsT = V chunk [k, 64] + ones column (row 64 accumulates the
    softmax denominator), rhs = probsT [k, q] -> [65, q] PSUM.
  - normalize: denominator row -> SBUF, reciprocal (fast-approx), GpSimd
    partition_broadcast, one VectorE multiply -> bf16 yt; per-(ft,qb)
    output flush.

Schedule (engine-balance aware). ScalarE exp is ~73us total and the PE's
real work is ~82us, so both must run dense from early on:
  - input DMAs are consolidated (adds, w-ft0 pair, xT quarters, the other
    w-fts, wv) so the first projections start a few us in; dummy matmuls
    bridge the HAM clock-gate ramp until data lands, and a dummy exp
    preloads the ACT spline table.
  - "wave A" computes Q/K ft0 kc-outer (4 PSUM accumulators round-robin),
    paced by the arriving xT quarters -> first score unit early.
  - the 8 score units run back-to-back; between exp pairs the PE pulls
    filler work from a deque fed by a per-unit plan: ft1, V, ft2, then
    AV blocks of done units interleaved ahead of ft3 so nothing misses
    its deadline and the tail stays short.
"""

import os
from collections import deque

import numpy as np
import ml_dtypes

import concourse.bass as bass
import concourse.tile as tile
from concourse import bacc
from concourse import mybir
from concourse.bass_utils import run_bass_kernel_spmd

B = 4
S = 1024
D = 1024
H = 16
HD = 64
NCORES = 8
FPG = 512          # features per head-group (8 heads * 64)
NKC = D // 128     # contraction chunks for projections
NFT = FPG // 128   # feature tiles per group
NST = S // 128     # sequence tiles
NQB = S // 512     # 512-wide query blocks
HPG = 8            # heads per group
NPAIR = NST // 2   # kt pairs per unit

BF16 = mybir.dt.bfloat16
F32 = mybir.dt.float32

_CACHE = {}


def _build():
    nc = bacc.Bacc()

    # Inputs pre-shuffled on host so HBM order matches SBUF order, and
    # consolidated so the priority path is few large DMAs.
    xT = nc.declare_dram_parameter("xT", [128, NKC, S], BF16, isOutput=False)
    adds = nc.declare_dram_parameter("adds", [128, 2, NFT], F32, isOutput=False)
    w0 = nc.declare_dram_parameter("w0", [128, 2, NKC, 128], BF16, isOutput=False)
    wqk = nc.declare_dram_parameter("wqk", [128, 2, 3, NKC, 128], BF16, isOutput=False)
    wvT = nc.declare_dram_parameter("wvT", [128, NKC, FPG], BF16, isOutput=False)
    # y blocks [ft, qb] of [128 feat, 512 q], bf16 (host re-expands to f32)
    yT = nc.declare_dram_parameter("yT", [NFT * NQB * 128, 512], BF16, isOutput=True)

    with tile.TileContext(nc) as tc:
        with (
            tc.tile_pool(name="persist", bufs=1) as persist,
            tc.tile_pool(name="probs", bufs=48) as probs_pool,
            tc.tile_pool(name="norm", bufs=3) as norm_pool,
            tc.tile_pool(name="psP", bufs=2, space=bass.MemorySpace.PSUM) as psP,
            tc.tile_pool(name="psA", bufs=2, space=bass.MemorySpace.PSUM) as psA,
            tc.tile_pool(name="psAV", bufs=2, space=bass.MemorySpace.PSUM) as psAV,
        ):
            # ---- persistent SBUF tensors ----
            xt = persist.tile([128, NKC, S], BF16)
            w0sb = persist.tile([128, 2, NKC, 128], BF16)      # wq/wk ft0
            wqksb = persist.tile([128, 2, 3, NKC, 128], BF16)  # wq/wk ft1-3
            wv = persist.tile([128, NKC, FPG], BF16)
            addsb = persist.tile([128, 2, NFT], F32)
            cq = persist.tile([128, NFT, S], BF16)          # cQT/8  [feat, S]
            ck = persist.tile([128, NFT, S], BF16)          # cKT    [feat, S]
            vt = persist.tile([128, NST, HPG, HD + 1], BF16)  # V' + ones col
            yt = persist.tile([128, NFT, S], BF16)          # yT [feat, S]
            wrm = persist.tile([128, 512], BF16)
            dum = persist.tile([1, 8], F32)

            def wsel(which, ft):
                wi = 0 if which == "q" else 1
                if ft == 0:
                    return w0sb[:, wi]
                return wqksb[:, wi, ft - 1]

            # ---- input DMAs, consolidated, priority order ----
            # (first xt pieces are small so wave A starts ASAP; the adds are
            # only needed by the evictions, so they ride behind)
            nc.sync.dma_start(out=w0sb[:], in_=w0[:])
            for lo, hi in ((0, 1), (1, 2), (2, 4), (4, 6), (6, 8)):
                nc.sync.dma_start(out=xt[:, lo:hi, :], in_=xT[:, lo:hi, :])
            nc.sync.dma_start(out=addsb[:], in_=adds[:])
            nc.sync.dma_start(out=wqksb[:], in_=wqk[:])
            nc.sync.dma_start(out=wv[:], in_=wvT[:])

            nc.vector.memset(wrm[:], 0.0)
            nc.vector.memset(vt[:, :, :, HD:HD + 1], 1.0)
            # preload the exp spline table while DMAs stream
            nc.scalar.activation(out=dum[:], in_=wrm[0:1, 0:8],
                                 func=mybir.ActivationFunctionType.Exp)

            # ---- HAM pre-warm: dummy matmuls until the first inputs land ----
            wacc = psAV.tile([128, 512], F32, tag="psAV")
            for _ in range(20):
                nc.tensor.matmul(
                    wacc[:], wrm[:, 0:128], wrm[:], start=True, stop=True
                )

            # ---- wave A: Q/K ft0, kc-outer, paced by the xT quarter DMAs ----
            accQ = psA.tile([128, 2, 512], F32, tag="psA")
            accK = psA.tile([128, 2, 512], F32, tag="psA")

            def wave_mm(acc, wi, kc, sb):
                nc.tensor.matmul(
                    acc[:, sb, :],
                    w0sb[:, wi, kc, :],
                    xt[:, kc, sb * 512:(sb + 1) * 512],
                    start=(kc == 0),
                    stop=(kc == NKC - 1),
                )

            def wave_evict(acc, wi, sb, lo=0, hi=512):
                dst, scale = (cq, 0.125) if wi == 0 else (ck, 1.0)
                nc.vector.tensor_scalar(
                    out=dst[:, 0, sb * 512 + lo:sb * 512 + hi],
                    in0=acc[:, sb, lo:hi],
                    scalar1=scale, scalar2=addsb[:, wi, 0:1],
                    op0=mybir.AluOpType.mult, op1=mybir.AluOpType.add,
                )

            for kc in range(NKC - 1):
                for acc, wi in ((accQ, 0), (accK, 1)):
                    for sb in range(NQB):
                        wave_mm(acc, wi, kc, sb)
            # last chunk: interleave evictions so the first score pair
            # (needs ck[kt0-1] + cq sb0) starts as early as possible; the
            # first ck eviction is split so its leading half lands sooner
            wave_mm(accK, 1, NKC - 1, 0)
            wave_evict(accK, 1, 0, 0, 256)
            wave_mm(accQ, 0, NKC - 1, 0)
            wave_evict(accQ, 0, 0)
            wave_mm(accK, 1, NKC - 1, 1)
            wave_evict(accK, 1, 0, 256, 512)
            wave_evict(accK, 1, 1)
            wave_mm(accQ, 0, NKC - 1, 1)
            wave_evict(accQ, 0, 1)

            # ---- filler deque: (pe_credit, op) ----
            fill = deque()

            def qk_group(which, ft, sb):
                wi = 0 if which == "q" else 1
                scale = 0.125 if which == "q" else 1.0
                dst = cq if which == "q" else ck
                w_ap = wsel(which, ft)
                acc = psP.tile([128, 512], F32, tag="psP", name="acc")
                for kc in range(NKC):
                    yield 1, (lambda kc=kc, acc=acc, w_ap=w_ap: nc.tensor.matmul(
                        acc[:],
                        w_ap[:, kc],
                        xt[:, kc, sb * 512:(sb + 1) * 512],
                        start=(kc == 0),
                        stop=(kc == NKC - 1),
                    ))
                yield 0, (lambda acc=acc: nc.vector.tensor_scalar(
                    out=dst[:, ft, sb * 512:(sb + 1) * 512],
                    in0=acc[:],
                    scalar1=scale,
                    scalar2=addsb[:, wi, ft:ft + 1],
                    op0=mybir.AluOpType.mult,
                    op1=mybir.AluOpType.add,
                ))

            def v_group(st):
                acc = psP.tile([128, 512], F32, tag="psP", name="acc")
                for kc in range(NKC):
                    yield 1, (lambda kc=kc, acc=acc: nc.tensor.matmul(
                        acc[:],
                        xt[:, kc, st * 128:(st + 1) * 128],
                        wv[:, kc, :],
                        start=(kc == 0),
                        stop=(kc == NKC - 1),
                    ))
                yield 0, (lambda acc=acc: nc.vector.tensor_copy(
                    out=vt[:, st, :, 0:HD], in_=acc[:]
                ))

            def head_av_mms(h, qb, pairs, av):
                for kt in range(NST):
                    yield 1, (lambda kt=kt, av=av: nc.tensor.matmul(
                        av[:],
                        vt[:, kt, h, :],
                        pairs[kt // 2][:, kt % 2, :],
                        start=(kt == 0),
                        stop=(kt == NST - 1),
                    ))

            def head_recb(av, den_on_act=False):
                """den -> reciprocal -> partition-broadcast (DVE+GpSimd).
                Late blocks copy the denominator on ScalarE (idle once the
                exps are done) so the DVE queue doesn't serialize the tail."""
                den = norm_pool.tile([1, 512], F32, tag="den", name="den")
                if den_on_act:
                    yield 0, (lambda av=av, den=den: nc.scalar.copy(
                        den[:], av[HD:HD + 1, :]
                    ))
                else:
                    yield 0, (lambda av=av, den=den: nc.vector.tensor_copy(
                        out=den[:], in_=av[HD:HD + 1, :]
                    ))
                rec = norm_pool.tile([1, 512], F32, tag="rec", name="rec")
                yield 0, (lambda den=den, rec=rec: nc.vector.reciprocal_approx_fast(
                    out=rec[:], in_=den[:]
                ))
                recb = norm_pool.tile([HD, 512], F32, tag="recb", name="recb")
                yield 0, (lambda rec=rec, recb=recb: nc.gpsimd.partition_broadcast(
                    recb[:], rec[:]
                ))
                return recb

            def norm_pair(hp, qb, avA, avB, den_on_act=False):
                """Both heads' den/recip/broadcast first, then the final
                multiplies + flushes -- keeps the DVE queue from stalling
                on the GpSimd broadcast latency."""
                qsl = slice(qb * 512, (qb + 1) * 512)
                recbs = []
                for av in (avA, avB):
                    g = head_recb(av, den_on_act)
                    while True:
                        try:
                            yield next(g)
                        except StopIteration as e:
                            recbs.append(e.value)
                            break
                for half, (av, recb) in enumerate(zip((avA, avB), recbs)):
                    pbase = half * 64
                    yield 0, (lambda av=av, recb=recb, pbase=pbase:
                              nc.vector.tensor_tensor(
                                  out=yt[pbase:pbase + 64, hp, qsl],
                                  in0=av[0:HD, :],
                                  in1=recb[:],
                                  op=mybir.AluOpType.mult,
                              ))
                    row = (hp * NQB + qb) * 128 + pbase
                    yield 0, (lambda row=row, pbase=pbase:
                              nc.sync.dma_start(
                                  out=yT[row:row + 64, :],
                                  in_=yt[pbase:pbase + 64, hp, qsl],
                              ))

            def av_mms(hp, qb, pairsA, pairsB):
                # both heads' matmul streams back-to-back (PE dense); PSUM
                # banks alternate pools (A->psAV, B->psP) so reuse is two
                # blocks apart and the norm chain never stalls the PE.
                avA = psAV.tile([HD + 1, 512], F32, tag="psAV", name="avA")
                yield from head_av_mms(2 * hp, qb, pairsA, avA)
                avB = psP.tile([HD + 1, 512], F32, tag="psP", name="avB")
                yield from head_av_mms(2 * hp + 1, qb, pairsB, avB)
                return avA, avB

            def av_block(hp, qb, pairsA, pairsB, den_on_act=False):
                g = av_mms(hp, qb, pairsA, pairsB)
                while True:
                    try:
                        yield next(g)
                    except StopIteration as e:
                        avA, avB = e.value
                        break
                yield from norm_pair(hp, qb, avA, avB, den_on_act)

            def pull(credits):
                got = 0
                while fill and got < credits:
                    cr, fn = fill.popleft()
                    fn()
                    got += cr

            # per-unit filler plan (each entry = ~32 PE credits):
            #   u0: QK ft1    u1: V 0-3     u2: V 4-7     u3: QK ft2
            #   u4: QK ft3    u5: AV(u0,u1) u6: AV(u2,u3) u7: AV(u4,u5)
            # drain: AV(u6), AV(u7) -- matmuls first, then pipelined norms
            def proj_segment(ft):
                for which in ("q", "k"):
                    for sb in range(NQB):
                        yield from qk_group(which, ft, sb)

            def v_segment(lo, hi):
                for st in range(lo, hi):
                    yield from v_group(st)

            units = [(hp, qb) for hp in range(HPG // 2) for qb in range(NQB)]
            pairs_of = {}
            av_sched = {5: [0, 1], 6: [2, 3], 7: [4, 5, 6]}

            def push_av(uidx, den_on_act=False):
                hp, qb = units[uidx]
                pp = pairs_of.pop(units[uidx])
                fill.extend(av_block(hp, qb, pp[0], pp[1], den_on_act))

            for ui, (hp, qb) in enumerate(units):
                if ui == 0:
                    fill.extend(proj_segment(1))
                elif ui == 1:
                    fill.extend(v_segment(0, 4))
                elif ui == 2:
                    fill.extend(v_segment(4, 8))
                elif ui == 3:
                    fill.extend(proj_segment(2))
                elif ui == 4:
                    fill.extend(proj_segment(3))
                for u in av_sched.get(ui, ()):
                    push_av(u, den_on_act=(ui >= 7))

                ft = hp
                qsl = slice(qb * 512, (qb + 1) * 512)
                pA, pB = [], []
                for p in range(NPAIR):
                    scA = psA.tile([128, 2, 512], F32, tag="psA", name="scA")
                    scB = psA.tile([128, 2, 512], F32, tag="psA", name="scB")
                    for j in range(2):
                        kt = 2 * p + j
                        ksl = slice(kt * 128, (kt + 1) * 128)
                        nc.tensor.matmul(
                            scA[:, j, :], ck[0:64, ft, ksl], cq[0:64, ft, qsl],
                            start=True, stop=True,
                        )
                        nc.tensor.matmul(
                            scB[:, j, :], ck[64:128, ft, ksl], cq[64:128, ft, qsl],
                            start=True, stop=True,
                        )
                    prA = probs_pool.tile([128, 2, 512], BF16, tag="probs", name="prA")
                    nc.scalar.activation(
                        out=prA[:], in_=scA[:],
                        func=mybir.ActivationFunctionType.Exp,
                    )
                    prB = probs_pool.tile([128, 2, 512], BF16, tag="probs", name="prB")
                    nc.scalar.activation(
                        out=prB[:], in_=scB[:],
                        func=mybir.ActivationFunctionType.Exp,
                    )
                    pA.append(prA)
                    pB.append(prB)
                    pull((12, 12, 12, 12)[p] if ui == 7 else (8, 10, 8, 6)[p])
                pairs_of[(hp, qb)] = (pA, pB)

            # drain: the last unit's own block; den copies ride the
            # now-idle ScalarE
            push_av(7, den_on_act=True)
            pull(10 ** 9)

    nc.finalize()
    return nc


def _get_nc():
    if "nc" not in _CACHE:
        _CACHE["nc"] = _build()
    return _CACHE["nc"]


def kernel(x, tokens, Wq, bq, Wk, bk, Wv, bv):
    x = np.asarray(x, dtype=np.float32)
    tokens = np.asarray(tokens, dtype=np.float32)
    Wq = np.asarray(Wq, dtype=np.float32)
    Wk = np.asarray(Wk, dtype=np.float32)
    Wv = np.asarray(Wv, dtype=np.float32)
    bq = np.asarray(bq, dtype=np.float32)
    bk = np.asarray(bk, dtype=np.float32)
    bv = np.asarray(bv, dtype=np.float32)

    bf16 = ml_dtypes.bfloat16
    in_maps = []
    for c in range(NCORES):
        b, g = divmod(c, 2)
        rows = slice(g * FPG, (g + 1) * FPG)
        tq = tokens[b, 0] @ Wq[rows].T + 2.0 * bq[rows]   # [512]
        tk = tokens[b, 0] @ Wk[rows].T + 2.0 * bk[rows]

        def packx(aT):
            # [D, C] -> [128, NKC, C]: partition-major to match SBUF layout
            return np.ascontiguousarray(
                aT.reshape(NKC, 128, aT.shape[1]).transpose(1, 0, 2)
            ).astype(bf16)

        def packw(aT):
            # [D, FPG] -> [128, NFT, NKC, 128]: ft-major
            return np.ascontiguousarray(
                aT.reshape(NKC, 128, NFT, 128).transpose(1, 2, 0, 3)
            ).astype(bf16)

        fq = packw(Wq[rows].T)
        fk = packw(Wk[rows].T)
        qa = (tq / 8.0).reshape(NFT, 128).T.astype(np.float32)   # [128, NFT]
        ka = tk.reshape(NFT, 128).T.astype(np.float32)

        in_maps.append({
            "xT": packx(x[b].T),
            "adds": np.ascontiguousarray(np.stack([qa, ka], axis=1)),
            "w0": np.ascontiguousarray(np.stack([fq[:, 0], fk[:, 0]], axis=1)),
            "wqk": np.ascontiguousarray(np.stack([fq[:, 1:], fk[:, 1:]], axis=1)),
            "wvT": packx(Wv[rows].T),
        })

    nc = _get_nc()
    trace = bool(int(os.environ.get("KERNEL_TRACE", "0")))
    res = run_bass_kernel_spmd(nc, in_maps, core_ids=list(range(NCORES)), trace=trace)
    if trace:
        _CACHE["last_results"] = res

    y = np.empty((B, S, D), dtype=np.float32)
    for c in range(NCORES):
        b, g = divmod(c, 2)
        blk = np.asarray(res.results[c]["yT"], dtype=np.float32)
        blk = blk.reshape(NFT, NQB, 128, 512)
        y[b, :, g * FPG:(g + 1) * FPG] = (
            blk.transpose(1, 3, 0, 2).reshape(S, FPG)
        )
    y += bv[None, None, :]
    return y


# revision 37
# speedup vs baseline: 1.0150x; 1.0150x over previous
"""Guide-token attention kernel for Trainium2 (8 NeuronCores).

Module: y[b] = softmax(((Q+tQ) @ (K+tK)^T)/sqrt(hd)) @ V  per head, where
  Q = x @ Wq^T + bq, K = x @ Wk^T + bk, V = x @ Wv^T + bv,
  tQ/tK are projections of a per-batch guide token (broadcast over seq).

Shapes: x [4, 1024, 1024], tokens [4, 1, 1024], W* [1024, 1024], b* [1024].
H=16 heads, hd=64.

Sharding: 8 cores = 4 batches x 2 head-groups (8 heads each); weights
column-sharded per head group; each core sees one batch -> no cross-core
communication.

Layout (PE contracts over the partition axis; no on-chip transposes):
  - host pre-transposes x[b] -> xT [D, S] and W slices (bf16), and
    precomputes the tiny guide-token adds (tq + 2*bq etc.).
  - QT/KT computed transposed [feat, S]; V computed natural [S, feat].
  - scores computed directly transposed per head: sT[k, q] = cK @ cQ^T
    (lhsT = cKT slice, rhs = cQT slice, contraction = hd = 64); the two
    heads of a pair live on PE row halves -> concurrent streams.
  - exp on ScalarE over two-bank PSUM tiles [128, 2, 512] -> bf16 probs.
    Softmax max-subtraction skipped: |scores| <= ~15, safe in fp32/bf16.
  - AV: lhsT = V chunk [k, 64] + ones column (row 64 accumulates the
    softmax denominator), rhs = probsT [k, q] -> [65, q] PSUM.
  - normalize: denominator row -> SBUF, reciprocal (fast-approx), GpSimd
    partition_broadcast, one VectorE multiply -> bf16 yt; per-(ft,qb)
    output flush.

Schedule (engine-balance aware). ScalarE exp is ~73us total and the PE's
real work is ~82us, so both must run dense from early on:
  - input DMAs are consolidated (adds, w-ft0 pair, xT quarters, the other
    w-fts, wv) so the first projections start a few us in; dummy matmuls
    bridge the HAM clock-gate ramp until data lands, and a dummy exp
    preloads the ACT spline table.
  - "wave A" computes Q/K ft0 kc-outer (4 PSUM accumulators round-robin),
    paced by the arriving xT quarters -> first score unit early.
  - the 8 score units run back-to-back; between exp pairs the PE pulls
    filler work from a deque fed by a per-unit plan: ft1, V, ft2, then
    AV blocks of done units interleaved ahead of ft3 so nothing misses
    its deadline and the tail stays short.
"""

import os
from collections import deque

import numpy as np
import ml_dtypes

import concourse.bass as bass
import concourse.tile as tile
from concourse import bacc
from concourse import mybir
from concourse.bass_utils import run_bass_kernel_spmd

B = 4
S = 1024
D = 1024
H = 16
HD = 64
NCORES = 8
FPG = 512          # features per head-group (8 heads * 64)
NKC = D // 128     # contraction chunks for projections
NFT = FPG // 128   # feature tiles per group
NST = S // 128     # sequence tiles
NQB = S // 512     # 512-wide query blocks
HPG = 8            # heads per group
NPAIR = NST // 2   # kt pairs per unit

BF16 = mybir.dt.bfloat16
F32 = mybir.dt.float32

_CACHE = {}


def _build():
    nc = bacc.Bacc()

    # Inputs pre-shuffled on host so HBM order matches SBUF order, and
    # consolidated so the priority path is few large DMAs.
    xT = nc.declare_dram_parameter("xT", [128, NKC, S], BF16, isOutput=False)
    adds = nc.declare_dram_parameter("adds", [128, 2, NFT], F32, isOutput=False)
    w0 = nc.declare_dram_parameter("w0", [128, 2, NKC, 128], BF16, isOutput=False)
    wqk = nc.declare_dram_parameter("wqk", [128, 2, 3, NKC, 128], BF16, isOutput=False)
    wvT = nc.declare_dram_parameter("wvT", [128, NKC, FPG], BF16, isOutput=False)
    # y blocks [ft, qb] of [128 feat, 512 q], bf16 (host re-expands to f32)
    yT = nc.declare_dram_parameter("yT", [NFT * NQB * 128, 512], BF16, isOutput=True)

    with tile.TileContext(nc) as tc:
        with (
            tc.tile_pool(name="persist", bufs=1) as persist,
            tc.tile_pool(name="probs", bufs=48) as probs_pool,
            tc.tile_pool(name="norm", bufs=3) as norm_pool,
            tc.tile_pool(name="psP", bufs=2, space=bass.MemorySpace.PSUM) as psP,
            tc.tile_pool(name="psA", bufs=2, space=bass.MemorySpace.PSUM) as psA,
            tc.tile_pool(name="psAV", bufs=2, space=bass.MemorySpace.PSUM) as psAV,
        ):
            # ---- persistent SBUF tensors ----
            xt = persist.tile([128, NKC, S], BF16)
            w0sb = persist.tile([128, 2, NKC, 128], BF16)      # wq/wk ft0
            wqksb = persist.tile([128, 2, 3, NKC, 128], BF16)  # wq/wk ft1-3
            wv = persist.tile([128, NKC, FPG], BF16)
            addsb = persist.tile([128, 2, NFT], F32)
            cq = persist.tile([128, NFT, S], BF16)          # cQT/8  [feat, S]
            ck = persist.tile([128, NFT, S], BF16)          # cKT    [feat, S]
            vt = persist.tile([128, NST, HPG, HD + 1], BF16)  # V' + ones col
            yt = persist.tile([128, NFT, S], BF16)          # yT [feat, S]
            wrm = persist.tile([128, 512], BF16)
            dum = persist.tile([1, 8], F32)

            def wsel(which, ft):
                wi = 0 if which == "q" else 1
                if ft == 0:
                    return w0sb[:, wi]
                return wqksb[:, wi, ft - 1]

            # ---- input DMAs, consolidated, priority order ----
            # (first xt pieces are small so wave A starts ASAP; the adds are
            # only needed by the evictions, so they ride behind)
            nc.sync.dma_start(out=w0sb[:], in_=w0[:])
            for lo, hi in ((0, 1), (1, 2), (2, 4), (4, 6), (6, 8)):
                nc.sync.dma_start(out=xt[:, lo:hi, :], in_=xT[:, lo:hi, :])
            nc.sync.dma_start(out=addsb[:], in_=adds[:])
            nc.sync.dma_start(out=wqksb[:], in_=wqk[:])
            nc.sync.dma_start(out=wv[:], in_=wvT[:])

            nc.vector.memset(wrm[:], 0.0)
            nc.vector.memset(vt[:, :, :, HD:HD + 1], 1.0)
            # preload the exp spline table while DMAs stream
            nc.scalar.activation(out=dum[:], in_=wrm[0:1, 0:8],
                                 func=mybir.ActivationFunctionType.Exp)

            # ---- HAM pre-warm: dummy matmuls until the first inputs land ----
            wacc = psAV.tile([128, 512], F32, tag="psAV")
            for _ in range(20):
                nc.tensor.matmul(
                    wacc[:], wrm[:, 0:128], wrm[:], start=True, stop=True
                )

            # ---- wave A: Q/K ft0, kc-outer, paced by the xT quarter DMAs ----
            accQ = psA.tile([128, 2, 512], F32, tag="psA")
            accK = psA.tile([128, 2, 512], F32, tag="psA")

            def wave_mm(acc, wi, kc, sb):
                nc.tensor.matmul(
                    acc[:, sb, :],
                    w0sb[:, wi, kc, :],
                    xt[:, kc, sb * 512:(sb + 1) * 512],
                    start=(kc == 0),
                    stop=(kc == NKC - 1),
                )

            def wave_evict(acc, wi, sb):
                dst, scale = (cq, 0.125) if wi == 0 else (ck, 1.0)
                nc.vector.tensor_scalar(
                    out=dst[:, 0, sb * 512:(sb + 1) * 512], in0=acc[:, sb, :],
                    scalar1=scale, scalar2=addsb[:, wi, 0:1],
                    op0=mybir.AluOpType.mult, op1=mybir.AluOpType.add,
                )

            for kc in range(NKC - 1):
                for acc, wi in ((accQ, 0), (accK, 1)):
                    for sb in range(NQB):
                        wave_mm(acc, wi, kc, sb)
            # last chunk: interleave evictions so the first score pair
            # (needs ck sb0 + cq sb0) starts as early as possible
            for acc, wi, sb in ((accK, 1, 0), (accQ, 0, 0), (accK, 1, 1), (accQ, 0, 1)):
                wave_mm(acc, wi, NKC - 1, sb)
                wave_evict(acc, wi, sb)

            # ---- filler deque: (pe_credit, op) ----
            fill = deque()

            def qk_group(which, ft, sb):
                wi = 0 if which == "q" else 1
                scale = 0.125 if which == "q" else 1.0
                dst = cq if which == "q" else ck
                w_ap = wsel(which, ft)
                acc = psP.tile([128, 512], F32, tag="psP", name="acc")
                for kc in range(NKC):
                    yield 1, (lambda kc=kc, acc=acc, w_ap=w_ap: nc.tensor.matmul(
                        acc[:],
                        w_ap[:, kc],
                        xt[:, kc, sb * 512:(sb + 1) * 512],
                        start=(kc == 0),
                        stop=(kc == NKC - 1),
                    ))
                yield 0, (lambda acc=acc: nc.vector.tensor_scalar(
                    out=dst[:, ft, sb * 512:(sb + 1) * 512],
                    in0=acc[:],
                    scalar1=scale,
                    scalar2=addsb[:, wi, ft:ft + 1],
                    op0=mybir.AluOpType.mult,
                    op1=mybir.AluOpType.add,
                ))

            def v_group(st):
                acc = psP.tile([128, 512], F32, tag="psP", name="acc")
                for kc in range(NKC):
                    yield 1, (lambda kc=kc, acc=acc: nc.tensor.matmul(
                        acc[:],
                        xt[:, kc, st * 128:(st + 1) * 128],
                        wv[:, kc, :],
                        start=(kc == 0),
                        stop=(kc == NKC - 1),
                    ))
                yield 0, (lambda acc=acc: nc.vector.tensor_copy(
                    out=vt[:, st, :, 0:HD], in_=acc[:]
                ))

            def head_av_mms(h, qb, pairs, av):
                for kt in range(NST):
                    yield 1, (lambda kt=kt, av=av: nc.tensor.matmul(
                        av[:],
                        vt[:, kt, h, :],
                        pairs[kt // 2][:, kt % 2, :],
                        start=(kt == 0),
                        stop=(kt == NST - 1),
                    ))

            def head_recb(av, den_on_act=False):
                """den -> reciprocal -> partition-broadcast (DVE+GpSimd).
                Late blocks copy the denominator on ScalarE (idle once the
                exps are done) so the DVE queue doesn't serialize the tail."""
                den = norm_pool.tile([1, 512], F32, tag="den", name="den")
                if den_on_act:
                    yield 0, (lambda av=av, den=den: nc.scalar.copy(
                        den[:], av[HD:HD + 1, :]
                    ))
                else:
                    yield 0, (lambda av=av, den=den: nc.vector.tensor_copy(
                        out=den[:], in_=av[HD:HD + 1, :]
                    ))
                rec = norm_pool.tile([1, 512], F32, tag="rec", name="rec")
                yield 0, (lambda den=den, rec=rec: nc.vector.reciprocal_approx_fast(
                    out=rec[:], in_=den[:]
                ))
                recb = norm_pool.tile([HD, 512], F32, tag="recb", name="recb")
                yield 0, (lambda rec=rec, recb=recb: nc.gpsimd.partition_broadcast(
                    recb[:], rec[:]
                ))
                return recb

            def norm_pair(hp, qb, avA, avB, den_on_act=False):
                """Both heads' den/recip/broadcast first, then the final
                multiplies + flushes -- keeps the DVE queue from stalling
                on the GpSimd broadcast latency."""
                qsl = slice(qb * 512, (qb + 1) * 512)
                recbs = []
                for av in (avA, avB):
                    g = head_recb(av, den_on_act)
                    while True:
                        try:
                            yield next(g)
                        except StopIteration as e:
                            recbs.append(e.value)
                            break
                for half, (av, recb) in enumerate(zip((avA, avB), recbs)):
                    pbase = half * 64
                    yield 0, (lambda av=av, recb=recb, pbase=pbase:
                              nc.vector.tensor_tensor(
                                  out=yt[pbase:pbase + 64, hp, qsl],
                                  in0=av[0:HD, :],
                                  in1=recb[:],
                                  op=mybir.AluOpType.mult,
                              ))
                    row = (hp * NQB + qb) * 128 + pbase
                    yield 0, (lambda row=row, pbase=pbase:
                              nc.sync.dma_start(
                                  out=yT[row:row + 64, :],
                                  in_=yt[pbase:pbase + 64, hp, qsl],
                              ))

            def av_mms(hp, qb, pairsA, pairsB):
                # both heads' matmul streams back-to-back (PE dense); PSUM
                # banks alternate pools (A->psAV, B->psP) so reuse is two
                # blocks apart and the norm chain never stalls the PE.
                avA = psAV.tile([HD + 1, 512], F32, tag="psAV", name="avA")
                yield from head_av_mms(2 * hp, qb, pairsA, avA)
                avB = psP.tile([HD + 1, 512], F32, tag="psP", name="avB")
                yield from head_av_mms(2 * hp + 1, qb, pairsB, avB)
                return avA, avB

            def av_block(hp, qb, pairsA, pairsB, den_on_act=False):
                g = av_mms(hp, qb, pairsA, pairsB)
                while True:
                    try:
                        yield next(g)
                    except StopIteration as e:
                        avA, avB = e.value
                        break
                yield from norm_pair(hp, qb, avA, avB, den_on_act)

            def pull(credits):
                got = 0
                while fill and got < credits:
                    cr, fn = fill.popleft()
                    fn()
                    got += cr

            # per-unit filler plan (each entry = ~32 PE credits):
            #   u0: QK ft1    u1: V 0-3     u2: V 4-7     u3: QK ft2
            #   u4: QK ft3    u5: AV(u0,u1) u6: AV(u2,u3) u7: AV(u4,u5)
            # drain: AV(u6), AV(u7) -- matmuls first, then pipelined norms
            def proj_segment(ft):
                for which in ("q", "k"):
                    for sb in range(NQB):
                        yield from qk_group(which, ft, sb)

            def v_segment(lo, hi):
                for st in range(lo, hi):
                    yield from v_group(st)

            units = [(hp, qb) for hp in range(HPG // 2) for qb in range(NQB)]
            pairs_of = {}
            av_sched = {5: [0, 1], 6: [2, 3], 7: [4, 5, 6]}

            def push_av(uidx, den_on_act=False):
                hp, qb = units[uidx]
                pp = pairs_of.pop(units[uidx])
                fill.extend(av_block(hp, qb, pp[0], pp[1], den_on_act))

            for ui, (hp, qb) in enumerate(units):
                if ui == 0:
                    fill.extend(proj_segment(1))
                elif ui == 1:
                    fill.extend(v_segment(0, 4))
                elif ui == 2:
                    fill.extend(v_segment(4, 8))
                elif ui == 3:
                    fill.extend(proj_segment(2))
                elif ui == 4:
                    fill.extend(proj_segment(3))
                for u in av_sched.get(ui, ()):
                    push_av(u, den_on_act=(ui >= 7))

                ft = hp
                qsl = slice(qb * 512, (qb + 1) * 512)
                pA, pB = [], []
                for p in range(NPAIR):
                    scA = psA.tile([128, 2, 512], F32, tag="psA", name="scA")
                    scB = psA.tile([128, 2, 512], F32, tag="psA", name="scB")
                    for j in range(2):
                        kt = 2 * p + j
                        ksl = slice(kt * 128, (kt + 1) * 128)
                        nc.tensor.matmul(
                            scA[:, j, :], ck[0:64, ft, ksl], cq[0:64, ft, qsl],
                            start=True, stop=True,
                        )
                        nc.tensor.matmul(
                            scB[:, j, :], ck[64:128, ft, ksl], cq[64:128, ft, qsl],
                            start=True, stop=True,
                        )
                    prA = probs_pool.tile([128, 2, 512], BF16, tag="probs", name="prA")
                    nc.scalar.activation(
                        out=prA[:], in_=scA[:],
                        func=mybir.ActivationFunctionType.Exp,
                    )
                    prB = probs_pool.tile([128, 2, 512], BF16, tag="probs", name="prB")
                    nc.scalar.activation(
                        out=prB[:], in_=scB[:],
                        func=mybir.ActivationFunctionType.Exp,
                    )
                    pA.append(prA)
                    pB.append(prB)
                    pull((12, 12, 12, 12)[p] if ui == 7 else (8, 10, 8, 6)[p])
                pairs_of[(hp, qb)] = (pA, pB)

            # drain: the last unit's own block; den copies ride the
            # now-idle ScalarE
            push_av(7, den_on_act=True)
            pull(10 ** 9)

    nc.finalize()
    return nc


def _get_nc():
    if "nc" not in _CACHE:
        _CACHE["nc"] = _build()
    return _CACHE["nc"]


def kernel(x, tokens, Wq, bq, Wk, bk, Wv, bv):
    x = np.asarray(x, dtype=np.float32)
    tokens = np.asarray(tokens, dtype=np.float32)
    Wq = np.asarray(Wq, dtype=np.float32)
    Wk = np.asarray(Wk, dtype=np.float32)
    Wv = np.asarray(Wv, dtype=np.float32)
    bq = np.asarray(bq, dtype=np.float32)
    bk = np.asarray(bk, dtype=np.float32)
    bv = np.asarray(bv, dtype=np.float32)

    bf16 = ml_dtypes.bfloat16
    in_maps = []
    for c in range(NCORES):
        b, g = divmod(c, 2)
        rows = slice(g * FPG, (g + 1) * FPG)
        tq = tokens[b, 0] @ Wq[rows].T + 2.0 * bq[rows]   # [512]
        tk = tokens[b, 0] @ Wk[rows].T + 2.0 * bk[rows]

        def packx(aT):
            # [D, C] -> [128, NKC, C]: partition-major to match SBUF layout
            return np.ascontiguousarray(
                aT.reshape(NKC, 128, aT.shape[1]).transpose(1, 0, 2)
            ).astype(bf16)

        def packw(aT):
            # [D, FPG] -> [128, NFT, NKC, 128]: ft-major
            return np.ascontiguousarray(
                aT.reshape(NKC, 128, NFT, 128).transpose(1, 2, 0, 3)
            ).astype(bf16)

        fq = packw(Wq[rows].T)
        fk = packw(Wk[rows].T)
        qa = (tq / 8.0).reshape(NFT, 128).T.astype(np.float32)   # [128, NFT]
        ka = tk.reshape(NFT, 128).T.astype(np.float32)

        in_maps.append({
            "xT": packx(x[b].T),
            "adds": np.ascontiguousarray(np.stack([qa, ka], axis=1)),
            "w0": np.ascontiguousarray(np.stack([fq[:, 0], fk[:, 0]], axis=1)),
            "wqk": np.ascontiguousarray(np.stack([fq[:, 1:], fk[:, 1:]], axis=1)),
            "wvT": packx(Wv[rows].T),
        })

    nc = _get_nc()
    trace = bool(int(os.environ.get("KERNEL_TRACE", "0")))
    res = run_bass_kernel_spmd(nc, in_maps, core_ids=list(range(NCORES)), trace=trace)
    if trace:
        _CACHE["last_results"] = res

    y = np.empty((B, S, D), dtype=np.float32)
    for c in range(NCORES):
        b, g = divmod(c, 2)
        blk = np.asarray(res.results[c]["yT"], dtype=np.float32)
        blk = blk.reshape(NFT, NQB, 128, 512)
        y[b, :, g * FPG:(g + 1) * FPG] = (
            blk.transpose(1, 3, 0, 2).reshape(S, FPG)
        )
    y += bv[None, None, :]
    return y


# revision 41
# speedup vs baseline: 1.0183x; 1.0033x over previous
"""Guide-token attention kernel for Trainium2 (8 NeuronCores).

Module: y[b] = softmax(((Q+tQ) @ (K+tK)^T)/sqrt(hd)) @ V  per head, where
  Q = x @ Wq^T + bq, K = x @ Wk^T + bk, V = x @ Wv^T + bv,
  tQ/tK are projections of a per-batch guide token (broadcast over seq).

Shapes: x [4, 1024, 1024], tokens [4, 1, 1024], W* [1024, 1024], b* [1024].
H=16 heads, hd=64.

Sharding: 8 cores = 4 batches x 2 head-groups (8 heads each); weights
column-sharded per head group; each core sees one batch -> no cross-core
communication.

Layout (PE contracts over the partition axis; no on-chip transposes):
  - host pre-transposes x[b] -> xT [D, S] and W slices (bf16), and
    precomputes the tiny guide-token adds (tq + 2*bq etc.).
  - QT/KT computed transposed [feat, S]; V computed natural [S, feat].
  - scores computed directly transposed per head: sT[k, q] = cK @ cQ^T
    (lhsT = cKT slice, rhs = cQT slice, contraction = hd = 64); the two
    heads of a pair live on PE row halves -> concurrent streams.
  - exp on ScalarE over two-bank PSUM tiles [128, 2, 512] -> bf16 probs.
    Softmax max-subtraction skipped: |scores| <= ~15, safe in fp32/bf16.
  - AV: lhsT = V chunk [k, 64] + ones column (row 64 accumulates the
    softmax denominator), rhs = probsT [k, q] -> [65, q] PSUM.
  - normalize: denominator row -> SBUF, reciprocal (fast-approx), GpSimd
    partition_broadcast, one VectorE multiply -> bf16 yt; per-(ft,qb)
    output flush.

Schedule (engine-balance aware). ScalarE exp is ~73us total and the PE's
real work is ~82us, so both must run dense from early on:
  - input DMAs are consolidated (adds, w-ft0 pair, xT quarters, the other
    w-fts, wv) so the first projections start a few us in; dummy matmuls
    bridge the HAM clock-gate ramp until data lands, and a dummy exp
    preloads the ACT spline table.
  - "wave A" computes Q/K ft0 kc-outer (4 PSUM accumulators round-robin),
    paced by the arriving xT quarters -> first score unit early.
  - the 8 score units run back-to-back; between exp pairs the PE pulls
    filler work from a deque fed by a per-unit plan: ft1, V, ft2, then
    AV blocks of done units interleaved ahead of ft3 so nothing misses
    its deadline and the tail stays short.
"""

import os
from collections import deque

import numpy as np
import ml_dtypes

import concourse.bass as bass
import concourse.tile as tile
from concourse import bacc
from concourse import mybir
from concourse.bass_utils import run_bass_kernel_spmd

B = 4
S = 1024
D = 1024
H = 16
HD = 64
NCORES = 8
FPG = 512          # features per head-group (8 heads * 64)
NKC = D // 128     # contraction chunks for projections
NFT = FPG // 128   # feature tiles per group
NST = S // 128     # sequence tiles
NQB = S // 512     # 512-wide query blocks
HPG = 8            # heads per group
NPAIR = NST // 2   # kt pairs per unit

BF16 = mybir.dt.bfloat16
F32 = mybir.dt.float32

_CACHE = {}


def _build():
    nc = bacc.Bacc()

    # Inputs pre-shuffled on host so HBM order matches SBUF order, and
    # consolidated so the priority path is few large DMAs.
    xT = nc.declare_dram_parameter("xT", [128, NKC, S], BF16, isOutput=False)
    adds = nc.declare_dram_parameter("adds", [128, 2, NFT], F32, isOutput=False)
    w0 = nc.declare_dram_parameter("w0", [128, 2, NKC, 128], BF16, isOutput=False)
    wqk = nc.declare_dram_parameter("wqk", [128, 2, 3, NKC, 128], BF16, isOutput=False)
    wvT = nc.declare_dram_parameter("wvT", [128, NKC, FPG], BF16, isOutput=False)
    # y blocks [ft, qb] of [128 feat, 512 q], bf16 (host re-expands to f32)
    yT = nc.declare_dram_parameter("yT", [NFT * NQB * 128, 512], BF16, isOutput=True)

    with tile.TileContext(nc) as tc:
        with (
            tc.tile_pool(name="persist", bufs=1) as persist,
            tc.tile_pool(name="probs", bufs=48) as probs_pool,
            tc.tile_pool(name="norm", bufs=3) as norm_pool,
            tc.tile_pool(name="psP", bufs=2, space=bass.MemorySpace.PSUM) as psP,
            tc.tile_pool(name="psA", bufs=2, space=bass.MemorySpace.PSUM) as psA,
            tc.tile_pool(name="psAV", bufs=2, space=bass.MemorySpace.PSUM) as psAV,
        ):
            # ---- persistent SBUF tensors ----
            xt = persist.tile([128, NKC, S], BF16)
            w0sb = persist.tile([128, 2, NKC, 128], BF16)      # wq/wk ft0
            wqksb = persist.tile([128, 2, 3, NKC, 128], BF16)  # wq/wk ft1-3
            wv = persist.tile([128, NKC, FPG], BF16)
            addsb = persist.tile([128, 2, NFT], F32)
            cq = persist.tile([128, NFT, S], BF16)          # cQT/8  [feat, S]
            ck = persist.tile([128, NFT, S], BF16)          # cKT    [feat, S]
            vt = persist.tile([128, NST, HPG, HD + 1], BF16)  # V' + ones col
            yt = persist.tile([128, NFT, S], BF16)          # yT [feat, S]
            wrm = persist.tile([128, 512], BF16)
            dum = persist.tile([1, 8], F32)

            def wsel(which, ft):
                wi = 0 if which == "q" else 1
                if ft == 0:
                    return w0sb[:, wi]
                return wqksb[:, wi, ft - 1]

            # ---- input DMAs, consolidated, priority order ----
            # (first xt pieces are small so wave A starts ASAP; the adds are
            # only needed by the evictions, so they ride behind)
            nc.sync.dma_start(out=w0sb[:], in_=w0[:])
            for lo, hi in ((0, 1), (1, 2), (2, 4), (4, 6), (6, 8)):
                nc.sync.dma_start(out=xt[:, lo:hi, :], in_=xT[:, lo:hi, :])
            nc.sync.dma_start(out=addsb[:], in_=adds[:])
            nc.sync.dma_start(out=wqksb[:], in_=wqk[:])
            nc.sync.dma_start(out=wv[:], in_=wvT[:])

            nc.vector.memset(wrm[:], 0.0)
            nc.vector.memset(vt[:, :, :, HD:HD + 1], 1.0)
            # preload the exp spline table while DMAs stream
            nc.scalar.activation(out=dum[:], in_=wrm[0:1, 0:8],
                                 func=mybir.ActivationFunctionType.Exp)

            # ---- HAM pre-warm: dummy matmuls until the first inputs land ----
            wacc = psAV.tile([128, 512], F32, tag="psAV", bufs=1)
            for _ in range(20):
                nc.tensor.matmul(
                    wacc[:], wrm[:, 0:128], wrm[:], start=True, stop=True
                )

            # ---- wave A: Q/K ft0, kc-outer, paced by the xT quarter DMAs ----
            accQ = psA.tile([128, 2, 512], F32, tag="psA")
            accK = psA.tile([128, 2, 512], F32, tag="psA")

            def wave_mm(acc, wi, kc, sb):
                nc.tensor.matmul(
                    acc[:, sb, :],
                    w0sb[:, wi, kc, :],
                    xt[:, kc, sb * 512:(sb + 1) * 512],
                    start=(kc == 0),
                    stop=(kc == NKC - 1),
                )

            def wave_evict(acc, wi, sb):
                dst, scale = (cq, 0.125) if wi == 0 else (ck, 1.0)
                nc.vector.tensor_scalar(
                    out=dst[:, 0, sb * 512:(sb + 1) * 512], in0=acc[:, sb, :],
                    scalar1=scale, scalar2=addsb[:, wi, 0:1],
                    op0=mybir.AluOpType.mult, op1=mybir.AluOpType.add,
                )

            for kc in range(NKC - 1):
                for acc, wi in ((accQ, 0), (accK, 1)):
                    for sb in range(NQB):
                        wave_mm(acc, wi, kc, sb)
            # last chunk: interleave evictions so the first score pair
            # (needs ck sb0 + cq sb0) starts as early as possible
            for acc, wi, sb in ((accK, 1, 0), (accQ, 0, 0), (accK, 1, 1), (accQ, 0, 1)):
                wave_mm(acc, wi, NKC - 1, sb)
                wave_evict(acc, wi, sb)

            # ---- filler deque: (pe_credit, op) ----
            fill = deque()

            def qk_group(which, ft, sb):
                wi = 0 if which == "q" else 1
                scale = 0.125 if which == "q" else 1.0
                dst = cq if which == "q" else ck
                w_ap = wsel(which, ft)
                acc = psP.tile([128, 512], F32, tag="psP", name="acc")
                for kc in range(NKC):
                    yield 1, (lambda kc=kc, acc=acc, w_ap=w_ap: nc.tensor.matmul(
                        acc[:],
                        w_ap[:, kc],
                        xt[:, kc, sb * 512:(sb + 1) * 512],
                        start=(kc == 0),
                        stop=(kc == NKC - 1),
                    ))
                yield 0, (lambda acc=acc: nc.vector.tensor_scalar(
                    out=dst[:, ft, sb * 512:(sb + 1) * 512],
                    in0=acc[:],
                    scalar1=scale,
                    scalar2=addsb[:, wi, ft:ft + 1],
                    op0=mybir.AluOpType.mult,
                    op1=mybir.AluOpType.add,
                ))

            def v_group(st):
                acc = psP.tile([128, 512], F32, tag="psP", name="acc")
                for kc in range(NKC):
                    yield 1, (lambda kc=kc, acc=acc: nc.tensor.matmul(
                        acc[:],
                        xt[:, kc, st * 128:(st + 1) * 128],
                        wv[:, kc, :],
                        start=(kc == 0),
                        stop=(kc == NKC - 1),
                    ))
                yield 0, (lambda acc=acc: nc.vector.tensor_copy(
                    out=vt[:, st, :, 0:HD], in_=acc[:]
                ))

            def head_av_mms(h, qb, pairs, av):
                for kt in range(NST):
                    yield 1, (lambda kt=kt, av=av: nc.tensor.matmul(
                        av[:],
                        vt[:, kt, h, :],
                        pairs[kt // 2][:, kt % 2, :],
                        start=(kt == 0),
                        stop=(kt == NST - 1),
                    ))

            def head_recb(av, den_on_act=False):
                """den -> reciprocal -> partition-broadcast (DVE+GpSimd).
                Late blocks copy the denominator on ScalarE (idle once the
                exps are done) so the DVE queue doesn't serialize the tail."""
                den = norm_pool.tile([1, 512], F32, tag="den", name="den")
                if den_on_act:
                    yield 0, (lambda av=av, den=den: nc.scalar.copy(
                        den[:], av[HD:HD + 1, :]
                    ))
                else:
                    yield 0, (lambda av=av, den=den: nc.vector.tensor_copy(
                        out=den[:], in_=av[HD:HD + 1, :]
                    ))
                rec = norm_pool.tile([1, 512], F32, tag="rec", name="rec")
                yield 0, (lambda den=den, rec=rec: nc.vector.reciprocal_approx_fast(
                    out=rec[:], in_=den[:]
                ))
                recb = norm_pool.tile([HD, 512], F32, tag="recb", name="recb")
                yield 0, (lambda rec=rec, recb=recb: nc.gpsimd.partition_broadcast(
                    recb[:], rec[:]
                ))
                return recb

            def norm_pair(hp, qb, avA, avB, den_on_act=False):
                """Both heads' den/recip/broadcast first, then the final
                multiplies + flushes -- keeps the DVE queue from stalling
                on the GpSimd broadcast latency."""
                qsl = slice(qb * 512, (qb + 1) * 512)
                recbs = []
                for av in (avA, avB):
                    g = head_recb(av, den_on_act)
                    while True:
                        try:
                            yield next(g)
                        except StopIteration as e:
                            recbs.append(e.value)
                            break
                for half, (av, recb) in enumerate(zip((avA, avB), recbs)):
                    pbase = half * 64
                    yield 0, (lambda av=av, recb=recb, pbase=pbase:
                              nc.vector.tensor_tensor(
                                  out=yt[pbase:pbase + 64, hp, qsl],
                                  in0=av[0:HD, :],
                                  in1=recb[:],
                                  op=mybir.AluOpType.mult,
                              ))
                    row = (hp * NQB + qb) * 128 + pbase
                    yield 0, (lambda row=row, pbase=pbase:
                              nc.sync.dma_start(
                                  out=yT[row:row + 64, :],
                                  in_=yt[pbase:pbase + 64, hp, qsl],
                              ))

            def av_mms(hp, qb, pairsA, pairsB):
                # both heads' matmul streams back-to-back (PE dense); PSUM
                # banks alternate pools (A->psAV, B->psP) so reuse is two
                # blocks apart and the norm chain never stalls the PE.
                avA = psAV.tile([HD + 1, 512], F32, tag="psAV", bufs=1, name="avA")
                yield from head_av_mms(2 * hp, qb, pairsA, avA)
                avB = psP.tile([HD + 1, 512], F32, tag="psP", name="avB")
                yield from head_av_mms(2 * hp + 1, qb, pairsB, avB)
                return avA, avB

            def av_block(hp, qb, pairsA, pairsB, den_on_act=False):
                g = av_mms(hp, qb, pairsA, pairsB)
                while True:
                    try:
                        yield next(g)
                    except StopIteration as e:
                        avA, avB = e.value
                        break
                yield from norm_pair(hp, qb, avA, avB, den_on_act)

            def pull(credits):
                got = 0
                while fill and got < credits:
                    cr, fn = fill.popleft()
                    fn()
                    got += cr

            # per-unit filler plan (each entry = ~32 PE credits):
            #   u0: QK ft1    u1: V 0-3     u2: V 4-7     u3: QK ft2
            #   u4: QK ft3    u5: AV(u0,u1) u6: AV(u2,u3) u7: AV(u4,u5)
            # drain: AV(u6), AV(u7) -- matmuls first, then pipelined norms
            def proj_segment(ft):
                for which in ("q", "k"):
                    for sb in range(NQB):
                        yield from qk_group(which, ft, sb)

            def v_segment(lo, hi):
                for st in range(lo, hi):
                    yield from v_group(st)

            units = [(hp, qb) for hp in range(HPG // 2) for qb in range(NQB)]
            pairs_of = {}
            av_sched = {5: [0, 1], 6: [2, 3], 7: [4, 5, 6]}

            # In units 0-4 the psAV bank pair is idle (no AV blocks yet), so
            # every third score tile borrows it -- a 3-deep score ring lets
            # the PE run one more exp ahead and shrinks the ACT-gating
            # stalls at pair/unit boundaries.
            sc_alloc = [0]

            def score_tile(ui):
                n = sc_alloc[0]
                sc_alloc[0] += 1
                if ui < 5 and n % 3 == 2:
                    return psAV.tile([128, 2, 512], F32, tag="psAV", bufs=1,
                                     name="scX")
                return psA.tile([128, 2, 512], F32, tag="psA", name="scA")

            def push_av(uidx, den_on_act=False):
                hp, qb = units[uidx]
                pp = pairs_of.pop(units[uidx])
                fill.extend(av_block(hp, qb, pp[0], pp[1], den_on_act))

            for ui, (hp, qb) in enumerate(units):
                if ui == 0:
                    fill.extend(proj_segment(1))
                elif ui == 1:
                    fill.extend(v_segment(0, 4))
                elif ui == 2:
                    fill.extend(v_segment(4, 8))
                elif ui == 3:
                    fill.extend(proj_segment(2))
                elif ui == 4:
                    fill.extend(proj_segment(3))
                for u in av_sched.get(ui, ()):
                    push_av(u, den_on_act=(ui >= 7))

                ft = hp
                qsl = slice(qb * 512, (qb + 1) * 512)
                pA, pB = [], []
                for p in range(NPAIR):
                    scA = score_tile(ui)
                    scB = score_tile(ui)
                    for j in range(2):
                        kt = 2 * p + j
                        ksl = slice(kt * 128, (kt + 1) * 128)
                        nc.tensor.matmul(
                            scA[:, j, :], ck[0:64, ft, ksl], cq[0:64, ft, qsl],
                            start=True, stop=True,
                        )
                        nc.tensor.matmul(
                            scB[:, j, :], ck[64:128, ft, ksl], cq[64:128, ft, qsl],
                            start=True, stop=True,
                        )
                    prA = probs_pool.tile([128, 2, 512], BF16, tag="probs", name="prA")
                    nc.scalar.activation(
                        out=prA[:], in_=scA[:],
                        func=mybir.ActivationFunctionType.Exp,
                    )
                    prB = probs_pool.tile([128, 2, 512], BF16, tag="probs", name="prB")
                    nc.scalar.activation(
                        out=prB[:], in_=scB[:],
                        func=mybir.ActivationFunctionType.Exp,
                    )
                    pA.append(prA)
                    pB.append(prB)
                    pull((12, 12, 12, 12)[p] if ui == 7 else (8, 10, 8, 6)[p])
                pairs_of[(hp, qb)] = (pA, pB)

            # drain: the last unit's own block; den copies ride the
            # now-idle ScalarE
            push_av(7, den_on_act=True)
            pull(10 ** 9)

    nc.finalize()
    return nc


def _get_nc():
    if "nc" not in _CACHE:
        _CACHE["nc"] = _build()
    return _CACHE["nc"]


def kernel(x, tokens, Wq, bq, Wk, bk, Wv, bv):
    x = np.asarray(x, dtype=np.float32)
    tokens = np.asarray(tokens, dtype=np.float32)
    Wq = np.asarray(Wq, dtype=np.float32)
    Wk = np.asarray(Wk, dtype=np.float32)
    Wv = np.asarray(Wv, dtype=np.float32)
    bq = np.asarray(bq, dtype=np.float32)
    bk = np.asarray(bk, dtype=np.float32)
    bv = np.asarray(bv, dtype=np.float32)

    bf16 = ml_dtypes.bfloat16
    in_maps = []
    for c in range(NCORES):
        b, g = divmod(c, 2)
        rows = slice(g * FPG, (g + 1) * FPG)
        tq = tokens[b, 0] @ Wq[rows].T + 2.0 * bq[rows]   # [512]
        tk = tokens[b, 0] @ Wk[rows].T + 2.0 * bk[rows]

        def packx(aT):
            # [D, C] -> [128, NKC, C]: partition-major to match SBUF layout
            return np.ascontiguousarray(
                aT.reshape(NKC, 128, aT.shape[1]).transpose(1, 0, 2)
            ).astype(bf16)

        def packw(aT):
            # [D, FPG] -> [128, NFT, NKC, 128]: ft-major
            return np.ascontiguousarray(
                aT.reshape(NKC, 128, NFT, 128).transpose(1, 2, 0, 3)
            ).astype(bf16)

        fq = packw(Wq[rows].T)
        fk = packw(Wk[rows].T)
        qa = (tq / 8.0).reshape(NFT, 128).T.astype(np.float32)   # [128, NFT]
        ka = tk.reshape(NFT, 128).T.astype(np.float32)

        in_maps.append({
            "xT": packx(x[b].T),
            "adds": np.ascontiguousarray(np.stack([qa, ka], axis=1)),
            "w0": np.ascontiguousarray(np.stack([fq[:, 0], fk[:, 0]], axis=1)),
            "wqk": np.ascontiguousarray(np.stack([fq[:, 1:], fk[:, 1:]], axis=1)),
            "wvT": packx(Wv[rows].T),
        })

    nc = _get_nc()
    trace = bool(int(os.environ.get("KERNEL_TRACE", "0")))
    res = run_bass_kernel_spmd(nc, in_maps, core_ids=list(range(NCORES)), trace=trace)
    if trace:
        _CACHE["last_results"] = res

    y = np.empty((B, S, D), dtype=np.float32)
    for c in range(NCORES):
        b, g = divmod(c, 2)
        blk = np.asarray(res.results[c]["yT"], dtype=np.float32)
        blk = blk.reshape(NFT, NQB, 128, 512)
        y[b, :, g * FPG:(g + 1) * FPG] = (
            blk.transpose(1, 3, 0, 2).reshape(S, FPG)
        )
    y += bv[None, None, :]
    return y


# revision 44
# speedup vs baseline: 1.0242x; 1.0057x over previous
"""Guide-token attention kernel for Trainium2 (8 NeuronCores).

Module: y[b] = softmax(((Q+tQ) @ (K+tK)^T)/sqrt(hd)) @ V  per head, where
  Q = x @ Wq^T + bq, K = x @ Wk^T + bk, V = x @ Wv^T + bv,
  tQ/tK are projections of a per-batch guide token (broadcast over seq).

Shapes: x [4, 1024, 1024], tokens [4, 1, 1024], W* [1024, 1024], b* [1024].
H=16 heads, hd=64.

Sharding: 8 cores = 4 batches x 2 head-groups (8 heads each); weights
column-sharded per head group; each core sees one batch -> no cross-core
communication.

Layout (PE contracts over the partition axis; no on-chip transposes):
  - host pre-transposes x[b] -> xT [D, S] and W slices (bf16), and
    precomputes the tiny guide-token adds (tq + 2*bq etc.).
  - QT/KT computed transposed [feat, S]; V computed natural [S, feat].
  - scores computed directly transposed per head: sT[k, q] = cK @ cQ^T
    (lhsT = cKT slice, rhs = cQT slice, contraction = hd = 64); the two
    heads of a pair live on PE row halves -> concurrent streams.
  - exp on ScalarE over two-bank PSUM tiles [128, 2, 512] -> bf16 probs.
    Softmax max-subtraction skipped: |scores| <= ~15, safe in fp32/bf16.
  - AV: lhsT = V chunk [k, 64] + ones column (row 64 accumulates the
    softmax denominator), rhs = probsT [k, q] -> [65, q] PSUM.
  - normalize: denominator row -> SBUF, reciprocal (fast-approx), GpSimd
    partition_broadcast, one VectorE multiply -> bf16 yt; per-(ft,qb)
    output flush.

Schedule (engine-balance aware). ScalarE exp is ~73us total and the PE's
real work is ~82us, so both must run dense from early on:
  - input DMAs are consolidated (adds, w-ft0 pair, xT quarters, the other
    w-fts, wv) so the first projections start a few us in; dummy matmuls
    bridge the HAM clock-gate ramp until data lands, and a dummy exp
    preloads the ACT spline table.
  - "wave A" computes Q/K ft0 kc-outer (4 PSUM accumulators round-robin),
    paced by the arriving xT quarters -> first score unit early.
  - the 8 score units run back-to-back; between exp pairs the PE pulls
    filler work from a deque fed by a per-unit plan: ft1, V, ft2, then
    AV blocks of done units interleaved ahead of ft3 so nothing misses
    its deadline and the tail stays short.
"""

import os
from collections import deque

import numpy as np
import ml_dtypes

import concourse.bass as bass
import concourse.tile as tile
from concourse import bacc
from concourse import mybir
from concourse.bass_utils import run_bass_kernel_spmd

B = 4
S = 1024
D = 1024
H = 16
HD = 64
NCORES = 8
FPG = 512          # features per head-group (8 heads * 64)
NKC = D // 128     # contraction chunks for projections
NFT = FPG // 128   # feature tiles per group
NST = S // 128     # sequence tiles
NQB = S // 512     # 512-wide query blocks
HPG = 8            # heads per group
NPAIR = NST // 2   # kt pairs per unit

BF16 = mybir.dt.bfloat16
F32 = mybir.dt.float32

_CACHE = {}


def _build():
    nc = bacc.Bacc()

    # Inputs pre-shuffled on host so HBM order matches SBUF order, and
    # consolidated so the priority path is few large DMAs.
    xT = nc.declare_dram_parameter("xT", [128, NKC, S], BF16, isOutput=False)
    adds = nc.declare_dram_parameter("adds", [128, 2, NFT], F32, isOutput=False)
    w0 = nc.declare_dram_parameter("w0", [128, 2, NKC, 128], BF16, isOutput=False)
    wqk = nc.declare_dram_parameter("wqk", [128, 2, 3, NKC, 128], BF16, isOutput=False)
    wvT = nc.declare_dram_parameter("wvT", [128, NKC, FPG], BF16, isOutput=False)
    # y blocks [ft, qb] of [128 feat, 512 q], bf16 (host re-expands to f32)
    yT = nc.declare_dram_parameter("yT", [NFT * NQB * 128, 512], BF16, isOutput=True)

    with tile.TileContext(nc) as tc:
        with (
            tc.tile_pool(name="persist", bufs=1) as persist,
            tc.tile_pool(name="probs", bufs=48) as probs_pool,
            tc.tile_pool(name="norm", bufs=3) as norm_pool,
            tc.tile_pool(name="psP", bufs=2, space=bass.MemorySpace.PSUM) as psP,
            tc.tile_pool(name="psA", bufs=2, space=bass.MemorySpace.PSUM) as psA,
            tc.tile_pool(name="psAV", bufs=2, space=bass.MemorySpace.PSUM) as psAV,
        ):
            # ---- persistent SBUF tensors ----
            xt = persist.tile([128, NKC, S], BF16)
            w0sb = persist.tile([128, 2, NKC, 128], BF16)      # wq/wk ft0
            wqksb = persist.tile([128, 2, 3, NKC, 128], BF16)  # wq/wk ft1-3
            wv = persist.tile([128, NKC, FPG], BF16)
            addsb = persist.tile([128, 2, NFT], F32)
            cq = persist.tile([128, NFT, S], BF16)          # cQT/8  [feat, S]
            ck = persist.tile([128, NFT, S], BF16)          # cKT    [feat, S]
            vt = persist.tile([128, NST, HPG, HD + 1], BF16)  # V' + ones col
            yt = persist.tile([128, NFT, S], BF16)          # yT [feat, S]
            wrm = persist.tile([128, 512], BF16)
            dum = persist.tile([1, 8], F32)

            def wsel(which, ft):
                wi = 0 if which == "q" else 1
                if ft == 0:
                    return w0sb[:, wi]
                return wqksb[:, wi, ft - 1]

            # ---- input DMAs, consolidated, priority order ----
            # (first xt pieces are small so wave A starts ASAP; the adds are
            # only needed by the evictions, so they ride behind)
            nc.sync.dma_start(out=w0sb[:], in_=w0[:])
            for lo, hi in ((0, 1), (1, 2), (2, 4), (4, 6), (6, 8)):
                nc.sync.dma_start(out=xt[:, lo:hi, :], in_=xT[:, lo:hi, :])
            nc.sync.dma_start(out=addsb[:], in_=adds[:])
            nc.sync.dma_start(out=wqksb[:], in_=wqk[:])
            nc.sync.dma_start(out=wv[:], in_=wvT[:])

            nc.vector.memset(wrm[:], 0.0)
            nc.vector.memset(vt[:, :, :, HD:HD + 1], 1.0)
            # preload the exp spline table while DMAs stream
            nc.scalar.activation(out=dum[:], in_=wrm[0:1, 0:8],
                                 func=mybir.ActivationFunctionType.Exp)

            # ---- HAM pre-warm: dummy matmuls until the first inputs land ----
            wacc = psAV.tile([128, 512], F32, tag="psAV", bufs=1)
            for _ in range(20):
                nc.tensor.matmul(
                    wacc[:], wrm[:, 0:128], wrm[:], start=True, stop=True
                )

            # ---- wave A: Q/K ft0, kc-outer, paced by the xT quarter DMAs ----
            accQ = psA.tile([128, 2, 512], F32, tag="psA")
            accK = psA.tile([128, 2, 512], F32, tag="psA")

            def wave_mm(acc, wi, kc, sb):
                nc.tensor.matmul(
                    acc[:, sb, :],
                    w0sb[:, wi, kc, :],
                    xt[:, kc, sb * 512:(sb + 1) * 512],
                    start=(kc == 0),
                    stop=(kc == NKC - 1),
                )

            def wave_evict(acc, wi, sb):
                dst, scale = (cq, 0.125) if wi == 0 else (ck, 1.0)
                nc.vector.tensor_scalar(
                    out=dst[:, 0, sb * 512:(sb + 1) * 512], in0=acc[:, sb, :],
                    scalar1=scale, scalar2=addsb[:, wi, 0:1],
                    op0=mybir.AluOpType.mult, op1=mybir.AluOpType.add,
                )

            # Only the three blocks the first score pairs need (cq sb0,
            # ck sb0, ck sb1); Q sb1 is deferred to the filler deque, off
            # the first-exp critical path.
            for kc in range(NKC - 1):
                for acc, wi, sb in ((accQ, 0, 0), (accK, 1, 0), (accK, 1, 1)):
                    wave_mm(acc, wi, kc, sb)
            # last chunk: interleave evictions so the first score pair
            # starts as early as possible
            for acc, wi, sb in ((accK, 1, 0), (accQ, 0, 0), (accK, 1, 1)):
                wave_mm(acc, wi, NKC - 1, sb)
                wave_evict(acc, wi, sb)

            # ---- filler deque: (pe_credit, op) ----
            fill = deque()

            def qk_group(which, ft, sb):
                wi = 0 if which == "q" else 1
                scale = 0.125 if which == "q" else 1.0
                dst = cq if which == "q" else ck
                w_ap = wsel(which, ft)
                acc = psP.tile([128, 512], F32, tag="psP", name="acc")
                for kc in range(NKC):
                    yield 1, (lambda kc=kc, acc=acc, w_ap=w_ap: nc.tensor.matmul(
                        acc[:],
                        w_ap[:, kc],
                        xt[:, kc, sb * 512:(sb + 1) * 512],
                        start=(kc == 0),
                        stop=(kc == NKC - 1),
                    ))
                yield 0, (lambda acc=acc: nc.vector.tensor_scalar(
                    out=dst[:, ft, sb * 512:(sb + 1) * 512],
                    in0=acc[:],
                    scalar1=scale,
                    scalar2=addsb[:, wi, ft:ft + 1],
                    op0=mybir.AluOpType.mult,
                    op1=mybir.AluOpType.add,
                ))

            def v_group(st):
                acc = psP.tile([128, 512], F32, tag="psP", name="acc")
                for kc in range(NKC):
                    yield 1, (lambda kc=kc, acc=acc: nc.tensor.matmul(
                        acc[:],
                        xt[:, kc, st * 128:(st + 1) * 128],
                        wv[:, kc, :],
                        start=(kc == 0),
                        stop=(kc == NKC - 1),
                    ))
                yield 0, (lambda acc=acc: nc.vector.tensor_copy(
                    out=vt[:, st, :, 0:HD], in_=acc[:]
                ))

            def head_av_mms(h, qb, pairs, av):
                for kt in range(NST):
                    yield 1, (lambda kt=kt, av=av: nc.tensor.matmul(
                        av[:],
                        vt[:, kt, h, :],
                        pairs[kt // 2][:, kt % 2, :],
                        start=(kt == 0),
                        stop=(kt == NST - 1),
                    ))

            def head_recb(av, den_on_act=False):
                """den -> reciprocal -> partition-broadcast (DVE+GpSimd).
                Late blocks copy the denominator on ScalarE (idle once the
                exps are done) so the DVE queue doesn't serialize the tail."""
                den = norm_pool.tile([1, 512], F32, tag="den", name="den")
                if den_on_act:
                    yield 0, (lambda av=av, den=den: nc.scalar.copy(
                        den[:], av[HD:HD + 1, :]
                    ))
                else:
                    yield 0, (lambda av=av, den=den: nc.vector.tensor_copy(
                        out=den[:], in_=av[HD:HD + 1, :]
                    ))
                rec = norm_pool.tile([1, 512], F32, tag="rec", name="rec")
                yield 0, (lambda den=den, rec=rec: nc.vector.reciprocal_approx_fast(
                    out=rec[:], in_=den[:]
                ))
                recb = norm_pool.tile([HD, 512], F32, tag="recb", name="recb")
                yield 0, (lambda rec=rec, recb=recb: nc.gpsimd.partition_broadcast(
                    recb[:], rec[:]
                ))
                return recb

            def norm_pair(hp, qb, avA, avB, den_on_act=False):
                """Both heads' den/recip/broadcast first, then the final
                multiplies + flushes -- keeps the DVE queue from stalling
                on the GpSimd broadcast latency."""
                qsl = slice(qb * 512, (qb + 1) * 512)
                recbs = []
                for av in (avA, avB):
                    g = head_recb(av, den_on_act)
                    while True:
                        try:
                            yield next(g)
                        except StopIteration as e:
                            recbs.append(e.value)
                            break
                for half, (av, recb) in enumerate(zip((avA, avB), recbs)):
                    pbase = half * 64
                    yield 0, (lambda av=av, recb=recb, pbase=pbase:
                              nc.vector.tensor_tensor(
                                  out=yt[pbase:pbase + 64, hp, qsl],
                                  in0=av[0:HD, :],
                                  in1=recb[:],
                                  op=mybir.AluOpType.mult,
                              ))
                    row = (hp * NQB + qb) * 128 + pbase
                    yield 0, (lambda row=row, pbase=pbase:
                              nc.sync.dma_start(
                                  out=yT[row:row + 64, :],
                                  in_=yt[pbase:pbase + 64, hp, qsl],
                              ))

            def av_mms(hp, qb, pairsA, pairsB):
                # both heads' matmul streams back-to-back (PE dense); PSUM
                # banks alternate pools (A->psAV, B->psP) so reuse is two
                # blocks apart and the norm chain never stalls the PE.
                avA = psAV.tile([HD + 1, 512], F32, tag="psAV", bufs=1, name="avA")
                yield from head_av_mms(2 * hp, qb, pairsA, avA)
                avB = psP.tile([HD + 1, 512], F32, tag="psP", name="avB")
                yield from head_av_mms(2 * hp + 1, qb, pairsB, avB)
                return avA, avB

            def av_block(hp, qb, pairsA, pairsB, den_on_act=False):
                g = av_mms(hp, qb, pairsA, pairsB)
                while True:
                    try:
                        yield next(g)
                    except StopIteration as e:
                        avA, avB = e.value
                        break
                yield from norm_pair(hp, qb, avA, avB, den_on_act)

            def pull(credits):
                got = 0
                while fill and got < credits:
                    cr, fn = fill.popleft()
                    fn()
                    got += cr

            # per-unit filler plan (each entry = ~32 PE credits):
            #   u0: QK ft1    u1: V 0-3     u2: V 4-7     u3: QK ft2
            #   u4: QK ft3    u5: AV(u0,u1) u6: AV(u2,u3) u7: AV(u4,u5)
            # drain: AV(u6), AV(u7) -- matmuls first, then pipelined norms
            def proj_segment(ft):
                for which in ("q", "k"):
                    for sb in range(NQB):
                        yield from qk_group(which, ft, sb)

            def v_segment(lo, hi):
                for st in range(lo, hi):
                    yield from v_group(st)

            units = [(hp, qb) for hp in range(HPG // 2) for qb in range(NQB)]
            pairs_of = {}
            av_sched = {5: [0, 1], 6: [2, 3], 7: [4, 5, 6]}

            # In units 0-4 the psAV bank pair is idle (no AV blocks yet), so
            # every third score tile borrows it -- a 3-deep score ring lets
            # the PE run one more exp ahead and shrinks the ACT-gating
            # stalls at pair/unit boundaries.
            sc_alloc = [0]

            def score_tile(ui):
                n = sc_alloc[0]
                sc_alloc[0] += 1
                if ui < 5 and n % 3 == 2:
                    return psAV.tile([128, 2, 512], F32, tag="psAV", bufs=1,
                                     name="scX")
                return psA.tile([128, 2, 512], F32, tag="psA", name="scA")

            def push_av(uidx, den_on_act=False):
                hp, qb = units[uidx]
                pp = pairs_of.pop(units[uidx])
                fill.extend(av_block(hp, qb, pp[0], pp[1], den_on_act))

            for ui, (hp, qb) in enumerate(units):
                if ui == 0:
                    fill.extend(qk_group("q", 0, 1))   # deferred from wave A
                    fill.extend(proj_segment(1))
                elif ui == 1:
                    fill.extend(v_segment(0, 4))
                elif ui == 2:
                    fill.extend(v_segment(4, 8))
                elif ui == 3:
                    fill.extend(proj_segment(2))
                elif ui == 4:
                    fill.extend(proj_segment(3))
                for u in av_sched.get(ui, ()):
                    push_av(u, den_on_act=(ui >= 7))

                ft = hp
                qsl = slice(qb * 512, (qb + 1) * 512)
                pA, pB = [], []
                for p in range(NPAIR):
                    scA = score_tile(ui)
                    scB = score_tile(ui)
                    for j in range(2):
                        kt = 2 * p + j
                        ksl = slice(kt * 128, (kt + 1) * 128)
                        nc.tensor.matmul(
                            scA[:, j, :], ck[0:64, ft, ksl], cq[0:64, ft, qsl],
                            start=True, stop=True,
                        )
                        nc.tensor.matmul(
                            scB[:, j, :], ck[64:128, ft, ksl], cq[64:128, ft, qsl],
                            start=True, stop=True,
                        )
                    prA = probs_pool.tile([128, 2, 512], BF16, tag="probs", name="prA")
                    nc.scalar.activation(
                        out=prA[:], in_=scA[:],
                        func=mybir.ActivationFunctionType.Exp,
                    )
                    prB = probs_pool.tile([128, 2, 512], BF16, tag="probs", name="prB")
                    nc.scalar.activation(
                        out=prB[:], in_=scB[:],
                        func=mybir.ActivationFunctionType.Exp,
                    )
                    pA.append(prA)
                    pB.append(prB)
                    if ui == 7:
                        pull(12)
                    elif ui < 4:
                        pull((10, 10, 8, 8)[p])
                    else:
                        pull((8, 10, 8, 6)[p])
                pairs_of[(hp, qb)] = (pA, pB)

            # drain: the last unit's own block; den copies ride the
            # now-idle ScalarE
            push_av(7, den_on_act=True)
            pull(10 ** 9)

    nc.finalize()
    return nc


def _get_nc():
    if "nc" not in _CACHE:
        _CACHE["nc"] = _build()
    return _CACHE["nc"]


def kernel(x, tokens, Wq, bq, Wk, bk, Wv, bv):
    x = np.asarray(x, dtype=np.float32)
    tokens = np.asarray(tokens, dtype=np.float32)
    Wq = np.asarray(Wq, dtype=np.float32)
    Wk = np.asarray(Wk, dtype=np.float32)
    Wv = np.asarray(Wv, dtype=np.float32)
    bq = np.asarray(bq, dtype=np.float32)
    bk = np.asarray(bk, dtype=np.float32)
    bv = np.asarray(bv, dtype=np.float32)

    bf16 = ml_dtypes.bfloat16
    in_maps = []
    for c in range(NCORES):
        b, g = divmod(c, 2)
        rows = slice(g * FPG, (g + 1) * FPG)
        tq = tokens[b, 0] @ Wq[rows].T + 2.0 * bq[rows]   # [512]
        tk = tokens[b, 0] @ Wk[rows].T + 2.0 * bk[rows]

        def packx(aT):
            # [D, C] -> [128, NKC, C]: partition-major to match SBUF layout
            return np.ascontiguousarray(
                aT.reshape(NKC, 128, aT.shape[1]).transpose(1, 0, 2)
            ).astype(bf16)

        def packw(aT):
            # [D, FPG] -> [128, NFT, NKC, 128]: ft-major
            return np.ascontiguousarray(
                aT.reshape(NKC, 128, NFT, 128).transpose(1, 2, 0, 3)
            ).astype(bf16)

        fq = packw(Wq[rows].T)
        fk = packw(Wk[rows].T)
        qa = (tq / 8.0).reshape(NFT, 128).T.astype(np.float32)   # [128, NFT]
        ka = tk.reshape(NFT, 128).T.astype(np.float32)

        in_maps.append({
            "xT": packx(x[b].T),
            "adds": np.ascontiguousarray(np.stack([qa, ka], axis=1)),
            "w0": np.ascontiguousarray(np.stack([fq[:, 0], fk[:, 0]], axis=1)),
            "wqk": np.ascontiguousarray(np.stack([fq[:, 1:], fk[:, 1:]], axis=1)),
            "wvT": packx(Wv[rows].T),
        })

    nc = _get_nc()
    trace = bool(int(os.environ.get("KERNEL_TRACE", "0")))
    res = run_bass_kernel_spmd(nc, in_maps, core_ids=list(range(NCORES)), trace=trace)
    if trace:
        _CACHE["last_results"] = res

    y = np.empty((B, S, D), dtype=np.float32)
    for c in range(NCORES):
        b, g = divmod(c, 2)
        blk = np.asarray(res.results[c]["yT"], dtype=np.float32)
        blk = blk.reshape(NFT, NQB, 128, 512)
        y[b, :, g * FPG:(g + 1) * FPG] = (
            blk.transpose(1, 3, 0, 2).reshape(S, FPG)
        )
    y += bv[None, None, :]
    return y


# revision 46
# speedup vs baseline: 1.0361x; 1.0116x over previous
"""Guide-token attention kernel for Trainium2 (8 NeuronCores).

Module: y[b] = softmax(((Q+tQ) @ (K+tK)^T)/sqrt(hd)) @ V  per head, where
  Q = x @ Wq^T + bq, K = x @ Wk^T + bk, V = x @ Wv^T + bv,
  tQ/tK are projections of a per-batch guide token (broadcast over seq).

Shapes: x [4, 1024, 1024], tokens [4, 1, 1024], W* [1024, 1024], b* [1024].
H=16 heads, hd=64.

Sharding: 8 cores = 4 batches x 2 head-groups (8 heads each); weights
column-sharded per head group; each core sees one batch -> no cross-core
communication.

Layout (PE contracts over the partition axis; no on-chip transposes):
  - host pre-transposes x[b] -> xT [D, S] and W slices (bf16), and
    precomputes the tiny guide-token adds (tq + 2*bq etc.).
  - QT/KT computed transposed [feat, S]; V computed natural [S, feat].
  - scores computed directly transposed per head: sT[k, q] = cK @ cQ^T
    (lhsT = cKT slice, rhs = cQT slice, contraction = hd = 64); the two
    heads of a pair live on PE row halves -> concurrent streams.
  - exp on ScalarE over two-bank PSUM tiles [128, 2, 512] -> bf16 probs.
    Softmax max-subtraction skipped: |scores| <= ~15, safe in fp32/bf16.
  - AV: lhsT = V chunk [k, 64] + ones column (row 64 accumulates the
    softmax denominator), rhs = probsT [k, q] -> [65, q] PSUM.
  - normalize: denominator row -> SBUF, reciprocal (fast-approx), GpSimd
    partition_broadcast, one VectorE multiply -> bf16 yt; per-(ft,qb)
    output flush.

Schedule (engine-balance aware). ScalarE exp is ~73us total and the PE's
real work is ~82us, so both must run dense from early on:
  - input DMAs are consolidated (adds, w-ft0 pair, xT quarters, the other
    w-fts, wv) so the first projections start a few us in; dummy matmuls
    bridge the HAM clock-gate ramp until data lands, and a dummy exp
    preloads the ACT spline table.
  - "wave A" computes Q/K ft0 kc-outer (4 PSUM accumulators round-robin),
    paced by the arriving xT quarters -> first score unit early.
  - the 8 score units run back-to-back; between exp pairs the PE pulls
    filler work from a deque fed by a per-unit plan: ft1, V, ft2, then
    AV blocks of done units interleaved ahead of ft3 so nothing misses
    its deadline and the tail stays short.
"""

import os
from collections import deque

import numpy as np
import ml_dtypes

import concourse.bass as bass
import concourse.tile as tile
from concourse import bacc
from concourse import mybir
from concourse.bass_utils import run_bass_kernel_spmd

B = 4
S = 1024
D = 1024
H = 16
HD = 64
NCORES = 8
FPG = 512          # features per head-group (8 heads * 64)
NKC = D // 128     # contraction chunks for projections
NFT = FPG // 128   # feature tiles per group
NST = S // 128     # sequence tiles
NQB = S // 512     # 512-wide query blocks
HPG = 8            # heads per group
NPAIR = NST // 2   # kt pairs per unit

BF16 = mybir.dt.bfloat16
F32 = mybir.dt.float32

_CACHE = {}


def _build():
    nc = bacc.Bacc()

    # Inputs pre-shuffled on host so HBM order matches SBUF order, and
    # consolidated so the priority path is few large DMAs.
    xT = nc.declare_dram_parameter("xT", [128, NKC, S], BF16, isOutput=False)
    adds = nc.declare_dram_parameter("adds", [128, 2, NFT], F32, isOutput=False)
    w0 = nc.declare_dram_parameter("w0", [128, 2, NKC, 128], BF16, isOutput=False)
    wqk = nc.declare_dram_parameter("wqk", [128, 2, 3, NKC, 128], BF16, isOutput=False)
    wvT = nc.declare_dram_parameter("wvT", [128, NKC, FPG], BF16, isOutput=False)
    # y blocks [ft, qb] of [128 feat, 512 q], bf16 (host re-expands to f32)
    yT = nc.declare_dram_parameter("yT", [NFT * NQB * 128, 512], BF16, isOutput=True)

    with tile.TileContext(nc) as tc:
        with (
            tc.tile_pool(name="persist", bufs=1) as persist,
            tc.tile_pool(name="probs", bufs=48) as probs_pool,
            tc.tile_pool(name="norm", bufs=3) as norm_pool,
            tc.tile_pool(name="psP", bufs=2, space=bass.MemorySpace.PSUM) as psP,
            tc.tile_pool(name="psA", bufs=2, space=bass.MemorySpace.PSUM) as psA,
            tc.tile_pool(name="psAV", bufs=2, space=bass.MemorySpace.PSUM) as psAV,
        ):
            # ---- persistent SBUF tensors ----
            xt = persist.tile([128, NKC, S], BF16)
            w0sb = persist.tile([128, 2, NKC, 128], BF16)      # wq/wk ft0
            wqksb = persist.tile([128, 2, 3, NKC, 128], BF16)  # wq/wk ft1-3
            wv = persist.tile([128, NKC, FPG], BF16)
            addsb = persist.tile([128, 2, NFT], F32)
            cq = persist.tile([128, NFT, S], BF16)          # cQT/8  [feat, S]
            ck = persist.tile([128, NFT, S], BF16)          # cKT    [feat, S]
            vt = persist.tile([128, NST, HPG, HD + 1], BF16)  # V' + ones col
            yt = persist.tile([128, NFT, S], BF16)          # yT [feat, S]
            wrm = persist.tile([128, 512], BF16)
            dum = persist.tile([1, 8], F32)

            def wsel(which, ft):
                wi = 0 if which == "q" else 1
                if ft == 0:
                    return w0sb[:, wi]
                return wqksb[:, wi, ft - 1]

            # ---- input DMAs, consolidated, priority order ----
            # (first xt pieces are small so wave A starts ASAP; the adds are
            # only needed by the evictions, so they ride behind)
            nc.sync.dma_start(out=w0sb[:], in_=w0[:])
            for lo, hi in ((0, 1), (1, 2), (2, 4), (4, 6), (6, 8)):
                nc.sync.dma_start(out=xt[:, lo:hi, :], in_=xT[:, lo:hi, :])
            nc.sync.dma_start(out=addsb[:], in_=adds[:])
            nc.sync.dma_start(out=wqksb[:], in_=wqk[:])
            nc.sync.dma_start(out=wv[:], in_=wvT[:])

            nc.vector.memset(wrm[:], 0.0)
            nc.vector.memset(vt[:, :, :, HD:HD + 1], 1.0)
            # preload the exp spline table while DMAs stream
            nc.scalar.activation(out=dum[:], in_=wrm[0:1, 0:8],
                                 func=mybir.ActivationFunctionType.Exp)

            # ---- HAM pre-warm: dummy matmuls until the first inputs land ----
            wacc = psAV.tile([128, 512], F32, tag="psAV", bufs=1)
            for _ in range(20):
                nc.tensor.matmul(
                    wacc[:], wrm[:, 0:128], wrm[:], start=True, stop=True
                )

            # ---- wave A: Q/K ft0, kc-outer, paced by the xT quarter DMAs ----
            accQ = psA.tile([128, 2, 512], F32, tag="psA")
            accK = psA.tile([128, 2, 512], F32, tag="psA")

            def wave_mm(acc, wi, kc, sb):
                nc.tensor.matmul(
                    acc[:, sb, :],
                    w0sb[:, wi, kc, :],
                    xt[:, kc, sb * 512:(sb + 1) * 512],
                    start=(kc == 0),
                    stop=(kc == NKC - 1),
                )

            def wave_evict(acc, wi, sb):
                dst, scale = (cq, 0.125) if wi == 0 else (ck, 1.0)
                nc.vector.tensor_scalar(
                    out=dst[:, 0, sb * 512:(sb + 1) * 512], in0=acc[:, sb, :],
                    scalar1=scale, scalar2=addsb[:, wi, 0:1],
                    op0=mybir.AluOpType.mult, op1=mybir.AluOpType.add,
                )

            # Only the two blocks the first score pair needs (cq sb0,
            # ck sb0); K/Q sb1 are deferred to the filler deque front, off
            # the first-exp critical path (ck sb1 is needed 2 pairs in).
            for kc in range(NKC - 1):
                for acc, wi, sb in ((accQ, 0, 0), (accK, 1, 0)):
                    wave_mm(acc, wi, kc, sb)
            # last chunk: interleave evictions so the first score pair
            # starts as early as possible
            for acc, wi, sb in ((accK, 1, 0), (accQ, 0, 0)):
                wave_mm(acc, wi, NKC - 1, sb)
                wave_evict(acc, wi, sb)

            # ---- filler deque: (pe_credit, op) ----
            fill = deque()

            def qk_group(which, ft, sb):
                wi = 0 if which == "q" else 1
                scale = 0.125 if which == "q" else 1.0
                dst = cq if which == "q" else ck
                w_ap = wsel(which, ft)
                acc = psP.tile([128, 512], F32, tag="psP", name="acc")
                for kc in range(NKC):
                    yield 1, (lambda kc=kc, acc=acc, w_ap=w_ap: nc.tensor.matmul(
                        acc[:],
                        w_ap[:, kc],
                        xt[:, kc, sb * 512:(sb + 1) * 512],
                        start=(kc == 0),
                        stop=(kc == NKC - 1),
                    ))
                yield 0, (lambda acc=acc: nc.vector.tensor_scalar(
                    out=dst[:, ft, sb * 512:(sb + 1) * 512],
                    in0=acc[:],
                    scalar1=scale,
                    scalar2=addsb[:, wi, ft:ft + 1],
                    op0=mybir.AluOpType.mult,
                    op1=mybir.AluOpType.add,
                ))

            def v_group(st):
                acc = psP.tile([128, 512], F32, tag="psP", name="acc")
                for kc in range(NKC):
                    yield 1, (lambda kc=kc, acc=acc: nc.tensor.matmul(
                        acc[:],
                        xt[:, kc, st * 128:(st + 1) * 128],
                        wv[:, kc, :],
                        start=(kc == 0),
                        stop=(kc == NKC - 1),
                    ))
                yield 0, (lambda acc=acc: nc.vector.tensor_copy(
                    out=vt[:, st, :, 0:HD], in_=acc[:]
                ))

            def head_av_mms(h, qb, pairs, av):
                for kt in range(NST):
                    yield 1, (lambda kt=kt, av=av: nc.tensor.matmul(
                        av[:],
                        vt[:, kt, h, :],
                        pairs[kt // 2][:, kt % 2, :],
                        start=(kt == 0),
                        stop=(kt == NST - 1),
                    ))

            def head_recb(av, den_on_act=False):
                """den -> reciprocal -> partition-broadcast (DVE+GpSimd).
                Late blocks copy the denominator on ScalarE (idle once the
                exps are done) so the DVE queue doesn't serialize the tail."""
                den = norm_pool.tile([1, 512], F32, tag="den", name="den")
                if den_on_act:
                    yield 0, (lambda av=av, den=den: nc.scalar.copy(
                        den[:], av[HD:HD + 1, :]
                    ))
                else:
                    yield 0, (lambda av=av, den=den: nc.vector.tensor_copy(
                        out=den[:], in_=av[HD:HD + 1, :]
                    ))
                rec = norm_pool.tile([1, 512], F32, tag="rec", name="rec")
                yield 0, (lambda den=den, rec=rec: nc.vector.reciprocal_approx_fast(
                    out=rec[:], in_=den[:]
                ))
                recb = norm_pool.tile([HD, 512], F32, tag="recb", name="recb")
                yield 0, (lambda rec=rec, recb=recb: nc.gpsimd.partition_broadcast(
                    recb[:], rec[:]
                ))
                return recb

            def norm_pair(hp, qb, avA, avB, den_on_act=False):
                """Both heads' den/recip/broadcast first, then the final
                multiplies + flushes -- keeps the DVE queue from stalling
                on the GpSimd broadcast latency."""
                qsl = slice(qb * 512, (qb + 1) * 512)
                recbs = []
                for av in (avA, avB):
                    g = head_recb(av, den_on_act)
                    while True:
                        try:
                            yield next(g)
                        except StopIteration as e:
                            recbs.append(e.value)
                            break
                for half, (av, recb) in enumerate(zip((avA, avB), recbs)):
                    pbase = half * 64
                    yield 0, (lambda av=av, recb=recb, pbase=pbase:
                              nc.vector.tensor_tensor(
                                  out=yt[pbase:pbase + 64, hp, qsl],
                                  in0=av[0:HD, :],
                                  in1=recb[:],
                                  op=mybir.AluOpType.mult,
                              ))
                    row = (hp * NQB + qb) * 128 + pbase
                    yield 0, (lambda row=row, pbase=pbase:
                              nc.sync.dma_start(
                                  out=yT[row:row + 64, :],
                                  in_=yt[pbase:pbase + 64, hp, qsl],
                              ))

            def av_mms(hp, qb, pairsA, pairsB):
                # both heads' matmul streams back-to-back (PE dense); PSUM
                # banks alternate pools (A->psAV, B->psP) so reuse is two
                # blocks apart and the norm chain never stalls the PE.
                avA = psAV.tile([HD + 1, 512], F32, tag="psAV", bufs=1, name="avA")
                yield from head_av_mms(2 * hp, qb, pairsA, avA)
                avB = psP.tile([HD + 1, 512], F32, tag="psP", name="avB")
                yield from head_av_mms(2 * hp + 1, qb, pairsB, avB)
                return avA, avB

            def av_block(hp, qb, pairsA, pairsB, den_on_act=False):
                g = av_mms(hp, qb, pairsA, pairsB)
                while True:
                    try:
                        yield next(g)
                    except StopIteration as e:
                        avA, avB = e.value
                        break
                yield from norm_pair(hp, qb, avA, avB, den_on_act)

            def pull(credits):
                got = 0
                while fill and got < credits:
                    cr, fn = fill.popleft()
                    fn()
                    got += cr

            # per-unit filler plan (each entry = ~32 PE credits):
            #   u0: QK ft1    u1: V 0-3     u2: V 4-7     u3: QK ft2
            #   u4: QK ft3    u5: AV(u0,u1) u6: AV(u2,u3) u7: AV(u4,u5)
            # drain: AV(u6), AV(u7) -- matmuls first, then pipelined norms
            def proj_segment(ft):
                for which in ("q", "k"):
                    for sb in range(NQB):
                        yield from qk_group(which, ft, sb)

            def v_segment(lo, hi):
                for st in range(lo, hi):
                    yield from v_group(st)

            units = [(hp, qb) for hp in range(HPG // 2) for qb in range(NQB)]
            pairs_of = {}
            av_sched = {5: [0, 1], 6: [2, 3], 7: [4, 5, 6]}

            # In units 0-4 the psAV bank pair is idle (no AV blocks yet), so
            # every third score tile borrows it -- a 3-deep score ring lets
            # the PE run one more exp ahead and shrinks the ACT-gating
            # stalls at pair/unit boundaries.
            sc_alloc = [0]

            def score_tile(ui):
                n = sc_alloc[0]
                sc_alloc[0] += 1
                if ui < 5 and n % 3 == 2:
                    return psAV.tile([128, 2, 512], F32, tag="psAV", bufs=1,
                                     name="scX")
                return psA.tile([128, 2, 512], F32, tag="psA", name="scA")

            def push_av(uidx, den_on_act=False):
                hp, qb = units[uidx]
                pp = pairs_of.pop(units[uidx])
                fill.extend(av_block(hp, qb, pp[0], pp[1], den_on_act))

            for ui, (hp, qb) in enumerate(units):
                if ui == 0:
                    fill.extend(qk_group("k", 0, 1))   # deferred from wave A
                    fill.extend(qk_group("q", 0, 1))
                    fill.extend(proj_segment(1))
                elif ui == 1:
                    fill.extend(v_segment(0, 4))
                elif ui == 2:
                    fill.extend(v_segment(4, 8))
                elif ui == 3:
                    fill.extend(proj_segment(2))
                elif ui == 4:
                    fill.extend(proj_segment(3))
                for u in av_sched.get(ui, ()):
                    push_av(u, den_on_act=(ui >= 7))

                ft = hp
                qsl = slice(qb * 512, (qb + 1) * 512)
                pA, pB = [], []
                for p in range(NPAIR):
                    scA = score_tile(ui)
                    scB = score_tile(ui)
                    for j in range(2):
                        kt = 2 * p + j
                        ksl = slice(kt * 128, (kt + 1) * 128)
                        nc.tensor.matmul(
                            scA[:, j, :], ck[0:64, ft, ksl], cq[0:64, ft, qsl],
                            start=True, stop=True,
                        )
                        nc.tensor.matmul(
                            scB[:, j, :], ck[64:128, ft, ksl], cq[64:128, ft, qsl],
                            start=True, stop=True,
                        )
                    prA = probs_pool.tile([128, 2, 512], BF16, tag="probs", name="prA")
                    nc.scalar.activation(
                        out=prA[:], in_=scA[:],
                        func=mybir.ActivationFunctionType.Exp,
                    )
                    prB = probs_pool.tile([128, 2, 512], BF16, tag="probs", name="prB")
                    nc.scalar.activation(
                        out=prB[:], in_=scB[:],
                        func=mybir.ActivationFunctionType.Exp,
                    )
                    pA.append(prA)
                    pB.append(prB)
                    if ui == 7:
                        pull(12)
                    elif ui < 4:
                        pull((10, 10, 8, 8)[p])
                    else:
                        pull((8, 10, 8, 6)[p])
                pairs_of[(hp, qb)] = (pA, pB)

            # drain: the last unit's own block; den copies ride the
            # now-idle ScalarE
            push_av(7, den_on_act=True)
            pull(10 ** 9)

    nc.finalize()
    return nc


def _get_nc():
    if "nc" not in _CACHE:
        _CACHE["nc"] = _build()
    return _CACHE["nc"]


def kernel(x, tokens, Wq, bq, Wk, bk, Wv, bv):
    x = np.asarray(x, dtype=np.float32)
    tokens = np.asarray(tokens, dtype=np.float32)
    Wq = np.asarray(Wq, dtype=np.float32)
    Wk = np.asarray(Wk, dtype=np.float32)
    Wv = np.asarray(Wv, dtype=np.float32)
    bq = np.asarray(bq, dtype=np.float32)
    bk = np.asarray(bk, dtype=np.float32)
    bv = np.asarray(bv, dtype=np.float32)

    bf16 = ml_dtypes.bfloat16
    in_maps = []
    for c in range(NCORES):
        b, g = divmod(c, 2)
        rows = slice(g * FPG, (g + 1) * FPG)
        tq = tokens[b, 0] @ Wq[rows].T + 2.0 * bq[rows]   # [512]
        tk = tokens[b, 0] @ Wk[rows].T + 2.0 * bk[rows]

        def packx(aT):
            # [D, C] -> [128, NKC, C]: partition-major to match SBUF layout
            return np.ascontiguousarray(
                aT.reshape(NKC, 128, aT.shape[1]).transpose(1, 0, 2)
            ).astype(bf16)

        def packw(aT):
            # [D, FPG] -> [128, NFT, NKC, 128]: ft-major
            return np.ascontiguousarray(
                aT.reshape(NKC, 128, NFT, 128).transpose(1, 2, 0, 3)
            ).astype(bf16)

        fq = packw(Wq[rows].T)
        fk = packw(Wk[rows].T)
        qa = (tq / 8.0).reshape(NFT, 128).T.astype(np.float32)   # [128, NFT]
        ka = tk.reshape(NFT, 128).T.astype(np.float32)

        in_maps.append({
            "xT": packx(x[b].T),
            "adds": np.ascontiguousarray(np.stack([qa, ka], axis=1)),
            "w0": np.ascontiguousarray(np.stack([fq[:, 0], fk[:, 0]], axis=1)),
            "wqk": np.ascontiguousarray(np.stack([fq[:, 1:], fk[:, 1:]], axis=1)),
            "wvT": packx(Wv[rows].T),
        })

    nc = _get_nc()
    trace = bool(int(os.environ.get("KERNEL_TRACE", "0")))
    res = run_bass_kernel_spmd(nc, in_maps, core_ids=list(range(NCORES)), trace=trace)
    if trace:
        _CACHE["last_results"] = res

    y = np.empty((B, S, D), dtype=np.float32)
    for c in range(NCORES):
        b, g = divmod(c, 2)
        blk = np.asarray(res.results[c]["yT"], dtype=np.float32)
        blk = blk.reshape(NFT, NQB, 128, 512)
        y[b, :, g * FPG:(g + 1) * FPG] = (
            blk.transpose(1, 3, 0, 2).reshape(S, FPG)
        )
    y += bv[None, None, :]
    return y
